# revision 5
# baseline (speedup 1.0000x reference)
"""Trainium2 Bass kernel for sheaf Dirichlet energy (ConsistencyBasedLaplacianBuilder).

loss = sum_e || maps[rev(e)] @ x[tgt(e)] - maps[e] @ x[src(e)] ||_F^2

Strategy (edge parallelism across 8 NeuronCores):
  The reference edge set is symmetric: edge e < H (=E/2) has its reverse at
  e + H, so the loss equals 2 * sum_{e<H} ||maps[e+H] x[dst] - maps[e] x[src]||^2.
  Each core takes a contiguous slice of the H half-edges. The host lays the
  per-edge operands out as one sequential bf16 stream (pure indexing /
  layout: per edge the two 4x4 maps -- with the second negated via sign
  flip -- and the two gathered 4x16 x rows); every float multiply/add that
  produces the loss runs on device:
    prod[e,i,f,jj] = mc[e,i,jj] * xc[e,f,jj]        (DVE mult, jj innermost)
    dd4 = prod[..,0:4] + prod[..,4:8]               (DVE add)
    dd2 = dd4[..,0:2] + dd4[..,2:4]                 (DVE add)
    dd  = dd2[..,0]   + dd2[..,1]                   (Pool add)
    acc[e, g] += sum_if dd^2                        (ScalarE Square+accum)
  The jj-sum over the 8 concatenated [A | -B] columns forms the difference
  directly. bf16 keeps DVE in its 2x packed mode; accumulators are f32.
  Per-core partial sums are added on the host.
"""

import sys
import types

import numpy as np

sys.path.insert(0, "/opt/trn_rl_repo")

N = 50000
D = 4
F = 16
DF = D * F            # 64 floats per node row
E = 1600000
H = E // 2            # 800000 undirected pairs
NCORES = 8
EPC = H // NCORES     # 100000 half-edges per core

NT = 800              # tiles of 128 edges per core (800*128 = 102400 >= 100000)
EPC_PAD = NT * 128
GT = 32               # tiles per group (one fused op chain per group)
NG = NT // GT         # 50 groups
XC_COLS = GT * 128    # bf16 cols per group: xc [f, jj] per tile
MC_COLS = GT * 32     # bf16 cols per group: mc [i, jj] per tile
G_COLS = XC_COLS + MC_COLS


def _inject_axon_hooks():
    """The container's antenv lacks axon_hooks; provide it so NTFF tracing
    (used by test.py, harmless otherwise) can register."""
    if "antenv.axon_hooks" in sys.modules:
        return
    mod = types.ModuleType("antenv.axon_hooks")
    mod._hook = None

    def set_axon_ntff_profile_hook(h):
        mod._hook = h

    def get_axon_ntff_profile_hook():
        return mod._hook

    mod.set_axon_ntff_profile_hook = set_axon_ntff_profile_hook
    mod.get_axon_ntff_profile_hook = get_axon_ntff_profile_hook
    sys.modules["antenv.axon_hooks"] = mod


def _build_program(ncores=NCORES):
    import concourse.bacc as bacc
    import concourse.bass as bass
    import concourse.tile as tile
    from concourse import mybir

    AP = bass.AP
    f32 = mybir.dt.float32
    bf16 = mybir.dt.bfloat16
    Op = mybir.AluOpType
    Act = mybir.ActivationFunctionType
    ds = bass.ds

    nc = bacc.Bacc("TRN2", target_bir_lowering=False, debug=False,
                   num_devices=ncores)

    stream_d = nc.dram_tensor("stream", [128, NG * G_COLS], bf16,
                              kind="ExternalInput")
    loss_d = nc.dram_tensor("loss", [1, 1], f32, kind="ExternalOutput")

    with tile.TileContext(nc) as tc, \
         tc.tile_pool(name="persist", bufs=1) as pp:

        acc = pp.tile([128, NG], f32, tag="acc")

        # double-buffered group buffers
        st = [pp.tile([128, G_COLS], bf16, tag=f"st{b}", name=f"st{b}")
              for b in range(2)]
        prod = [pp.tile([128, GT * 512], bf16, tag=f"prod{b}", name=f"prod{b}")
                for b in range(2)]
        dd4 = [pp.tile([128, GT * 256], bf16, tag=f"dd4{b}", name=f"dd4{b}")
               for b in range(2)]
        dd2 = [pp.tile([128, GT * 128], bf16, tag=f"dd2{b}", name=f"dd2{b}")
               for b in range(2)]
        dd = [pp.tile([128, GT * 64], bf16, tag=f"dd{b}", name=f"dd{b}")
              for b in range(2)]
        sq = [pp.tile([128, GT * 64], bf16, tag=f"sq{b}", name=f"sq{b}")
              for b in range(2)]

        def load(g):
            b = g % 2
            nc.sync.dma_start(st[b][:], stream_d[:, ds(g * G_COLS, G_COLS)])

        def compute(g):
            b = g % 2
            xc = st[b][:, 0:XC_COLS]
            mc = st[b][:, XC_COLS:G_COLS]
            p = prod[b][:]
            # prod[t, i, f, jj] = xc[t, f, jj] * mc[t, i, (f), jj], one op
            # per i so every AP stays 3-free-dim (keeps DVE in 2x mode)
            in_x = AP(xc.tensor, xc.offset,
                      [xc.ap[0], [128, GT], [8, 16], [1, 8]])
            for i in range(4):
                out_i = AP(p.tensor, p.offset + 128 * i,
                           [p.ap[0], [512, GT], [8, 16], [1, 8]])
                in_m = AP(mc.tensor, mc.offset + 8 * i,
                          [mc.ap[0], [32, GT], [0, 16], [1, 8]])
                nc.vector.tensor_tensor(out_i, in_x, in_m, Op.mult)

            # dd4[(ti), f, jj4] = prod[.., 0:4] + prod[.., 4:8]
            a4 = dd4[b][:]
            pin0 = AP(p.tensor, p.offset,
                      [p.ap[0], [128, 4 * GT], [8, 16], [1, 4]])
            pin1 = AP(p.tensor, p.offset + 4,
                      [p.ap[0], [128, 4 * GT], [8, 16], [1, 4]])
            o4 = AP(a4.tensor, a4.offset,
                    [a4.ap[0], [64, 4 * GT], [4, 16], [1, 4]])
            nc.vector.tensor_tensor(o4, pin0, pin1, Op.add)

            # dd2 = dd4[.., 0:2] + dd4[.., 2:4]
            a2 = dd2[b][:]
            q0 = AP(a4.tensor, a4.offset,
                    [a4.ap[0], [64, 4 * GT], [4, 16], [1, 2]])
            q1 = AP(a4.tensor, a4.offset + 2,
                    [a4.ap[0], [64, 4 * GT], [4, 16], [1, 2]])
            o2 = AP(a2.tensor, a2.offset,
                    [a2.ap[0], [32, 4 * GT], [2, 16], [1, 2]])
            nc.vector.tensor_tensor(o2, q0, q1, Op.add)

            # dd = dd2[.., 0] + dd2[.., 1]  (Pool engine)
            a1 = dd[b][:]
            r0 = AP(a2.tensor, a2.offset,
                    [a2.ap[0], [32, 4 * GT], [2, 16]])
            r1 = AP(a2.tensor, a2.offset + 1,
                    [a2.ap[0], [32, 4 * GT], [2, 16]])
            o1 = AP(a1.tensor, a1.offset,
                    [a1.ap[0], [16, 4 * GT], [1, 16]])
            nc.gpsimd.tensor_tensor(o1, r0, r1, Op.add)

            # acc[:, g] = sum_if dd^2   (ScalarE)
            nc.scalar.activation(sq[b][:], a1, Act.Square,
                                 accum_out=acc[:, g:g + 1])

        load(0)
        for g in range(NG):
            if g + 1 < NG:
                load(g + 1)
            compute(g)

        colsum = pp.tile([128, 1], f32, tag="colsum")
        ones = pp.tile([128, 1], f32, tag="ones")
        nc.vector.reduce_sum(out=colsum[:], in_=acc[:],
                             axis=mybir.AxisListType.X)
        nc.gpsimd.memset(ones[:], 1.0)
        with tc.tile_pool(name="psum", bufs=1, space="PSUM") as psp:
            pt = psp.tile([1, 1], f32, tag="pt")
            nc.tensor.matmul(pt[:], lhsT=colsum[:], rhs=ones[:],
                             start=True, stop=True)
            lsb = pp.tile([1, 1], f32, tag="lsb")
            # *2: each undirected pair contributes both directed edges equally
            nc.vector.tensor_scalar(lsb[:], pt[:], 2.0, None, Op.mult)
            nc.sync.dma_start(loss_d[:], lsb[:])

    nc.compile()
    return nc


_CACHED = {}


def _get_program():
    if "nc" not in _CACHED:
        _inject_axon_hooks()
        _CACHED["nc"] = _build_program()
    return _CACHED["nc"]


def _prep_core_inputs(x_bf, mapsA_bf, mapsBn_bf, dst, src, core):
    """Build the per-core bf16 stream (indexing / layout only).

    x_bf:     [N, D, F] bf16 node features
    mapsA_bf: [H, D, D] bf16 = maps[H:] (the A = F_{v->u} map of pair e)
    mapsBn_bf:[H, D, D] bf16 = -maps[:H] (negated B, sign bit flipped)
    """
    import ml_dtypes
    BF = ml_dtypes.bfloat16

    e0 = core * EPC
    e1 = e0 + EPC

    di = np.zeros(EPC_PAD, np.int64)
    di[:EPC] = dst[e0:e1]
    si = np.zeros(EPC_PAD, np.int64)
    si[:EPC] = src[e0:e1]

    # xc[e, f, jj]: jj<4 -> x[dst][jj, f], jj>=4 -> x[src][jj-4, f]
    xc = np.concatenate([x_bf[di], x_bf[si]], axis=1)   # [P, 8(jj), 16(f)]
    xc = np.ascontiguousarray(xc.transpose(0, 2, 1))    # [P, f, jj]
    xc[EPC:] = 0
    xc = xc.reshape(NG, GT * 128, DF * 2)

    # mc[e, i, jj] = [A[i, :] | -B[i, :]]
    mc = np.zeros((EPC_PAD, D, 2 * D), BF)
    mc[:EPC, :, :D] = mapsA_bf[e0:e1]
    mc[:EPC, :, D:] = mapsBn_bf[e0:e1]
    mc = mc.reshape(NG, GT * 128, 4 * 2 * D)

    # per group: [xc tiles | mc tiles], tile-major [128, cols] per tile
    stream = np.empty((128, NG, G_COLS), BF)
    stream[:, :, :XC_COLS] = (
        xc.reshape(NG, GT, 128, 128).transpose(2, 0, 1, 3).reshape(128, NG, -1))
    stream[:, :, XC_COLS:] = (
        mc.reshape(NG, GT, 128, 32).transpose(2, 0, 1, 3).reshape(128, NG, -1))
    return {"stream": np.ascontiguousarray(stream.reshape(128, NG * G_COLS))}


def _prep_all_in_maps(x, restriction_maps, edge_index):
    import ml_dtypes
    BF = ml_dtypes.bfloat16

    x_bf = np.ascontiguousarray(x.reshape(N, D, F)).astype(BF)
    maps = np.asarray(restriction_maps)
    mapsA_bf = maps[H:].astype(BF)
    mapsBn_bf = (-maps[:H]).astype(BF)
    src = np.asarray(edge_index[0], np.int64)
    dst = np.asarray(edge_index[1], np.int64)
    return [_prep_core_inputs(x_bf, mapsA_bf, mapsBn_bf, dst, src, c)
            for c in range(NCORES)]


def _symmetric_structure(rev_idx):
    r = np.asarray(rev_idx)
    if r.shape != (E,):
        return False
    h = np.arange(H, dtype=r.dtype)
    return bool(np.array_equal(r[:H], h + H) and np.array_equal(r[H:], h))


def _fallback_numpy(x, restriction_maps, edge_index, rev_idx):
    x = np.asarray(x, np.float32)
    maps = np.asarray(restriction_maps, np.float32)
    ei = np.asarray(edge_index)
    rv = np.asarray(rev_idx)
    total = np.float64(0.0)
    chunk = 131072
    ne = ei.shape[1]
    for s in range(0, ne, chunk):
        e = min(s + chunk, ne)
        src = ei[0, s:e]
        tgt = ei[1, s:e]
        fvu = maps[rv[s:e]]
        fuv = maps[s:e]
        t1 = np.einsum("eij,ejf->eif", fvu, x[tgt])
        t2 = np.einsum("eij,ejf->eif", fuv, x[src])
        d = t1 - t2
        total += np.sum((d * d).astype(np.float64))
    return np.float32(total)


def kernel(x, restriction_maps, edge_index, rev_idx):
    x = np.asarray(x)
    restriction_maps = np.asarray(restriction_maps)
    edge_index = np.asarray(edge_index)
    rev_idx = np.asarray(rev_idx)

    if (x.shape != (N, D, F) or restriction_maps.shape != (E, D, D)
            or edge_index.shape != (2, E) or not _symmetric_structure(rev_idx)):
        return _fallback_numpy(x, restriction_maps, edge_index, rev_idx)

    from concourse.bass_utils import run_bass_kernel_spmd

    nc = _get_program()
    in_maps = _prep_all_in_maps(x, restriction_maps, edge_index)
    res = run_bass_kernel_spmd(nc, in_maps, core_ids=list(range(NCORES)))
    total = np.float32(0.0)
    for c in range(NCORES):
        total += res.results[c]["loss"][0, 0]
    return np.float32(total)


# revision 8
# speedup vs baseline: 1.0780x; 1.0780x over previous
"""Trainium2 Bass kernel for sheaf Dirichlet energy (ConsistencyBasedLaplacianBuilder).

loss = sum_e || maps[rev(e)] @ x[tgt(e)] - maps[e] @ x[src(e)] ||_F^2

Strategy (edge parallelism across 8 NeuronCores):
  The reference edge set is symmetric: edge e < H (=E/2) has its reverse at
  e + H, so the loss equals 2 * sum_{e<H} ||maps[e+H] x[dst] - maps[e] x[src]||^2.
  Each core takes a contiguous slice of the H half-edges. The host lays the
  per-edge operands out as one sequential bf16 stream (pure indexing /
  layout: per edge the two 4x4 maps -- with the second negated via sign
  flip -- and the two gathered 4x16 x rows); every float multiply/add that
  produces the loss runs on device:
    prod[e,i,f,jj] = mc[e,i,jj] * xc[e,f,jj]        (DVE mult, jj innermost)
    dd4 = prod[..,0:4] + prod[..,4:8]               (DVE add)
    dd2 = dd4[..,0:2] + dd4[..,2:4]                 (DVE add)
    dd  = dd2[..,0]   + dd2[..,1]                   (Pool add)
    acc[e, g] += sum_if dd^2                        (ScalarE Square+accum)
  The jj-sum over the 8 concatenated [A | -B] columns forms the difference
  directly. bf16 keeps DVE in its 2x packed mode; accumulators are f32.
  Per-core partial sums are added on the host.
"""

import sys
import types

import numpy as np

sys.path.insert(0, "/opt/trn_rl_repo")

N = 50000
D = 4
F = 16
DF = D * F            # 64 floats per node row
E = 1600000
H = E // 2            # 800000 undirected pairs
NCORES = 8
EPC = H // NCORES     # 100000 half-edges per core

NT = 800              # tiles of 128 edges per core (800*128 = 102400 >= 100000)
EPC_PAD = NT * 128
GT = 32               # tiles per group (one fused op chain per group)
NG = NT // GT         # 50 groups
XC_COLS = GT * 128    # bf16 cols per group: xc [f, jj] per tile
MC_COLS = GT * 32     # bf16 cols per group: mc [i, jj] per tile
G_COLS = XC_COLS + MC_COLS


def _inject_axon_hooks():
    """The container's antenv lacks axon_hooks; provide it so NTFF tracing
    (used by test.py, harmless otherwise) can register."""
    if "antenv.axon_hooks" in sys.modules:
        return
    mod = types.ModuleType("antenv.axon_hooks")
    mod._hook = None

    def set_axon_ntff_profile_hook(h):
        mod._hook = h

    def get_axon_ntff_profile_hook():
        return mod._hook

    mod.set_axon_ntff_profile_hook = set_axon_ntff_profile_hook
    mod.get_axon_ntff_profile_hook = get_axon_ntff_profile_hook
    sys.modules["antenv.axon_hooks"] = mod


def _build_program(ncores=NCORES):
    import concourse.bacc as bacc
    import concourse.bass as bass
    import concourse.tile as tile
    from concourse import mybir

    AP = bass.AP
    f32 = mybir.dt.float32
    bf16 = mybir.dt.bfloat16
    Op = mybir.AluOpType
    Act = mybir.ActivationFunctionType
    ds = bass.ds

    nc = bacc.Bacc("TRN2", target_bir_lowering=False, debug=False,
                   num_devices=ncores)

    stream_d = nc.dram_tensor("stream", [128, NG * G_COLS], bf16,
                              kind="ExternalInput")
    loss_d = nc.dram_tensor("loss", [1, 1], f32, kind="ExternalOutput")

    with tile.TileContext(nc) as tc, \
         tc.tile_pool(name="persist", bufs=1) as pp:

        acc = pp.tile([128, NG], f32, tag="acc")

        # double-buffered group buffers
        st = [pp.tile([128, G_COLS], bf16, tag=f"st{b}", name=f"st{b}")
              for b in range(2)]
        prod = [pp.tile([128, GT * 512], bf16, tag=f"prod{b}", name=f"prod{b}")
                for b in range(2)]
        dd4 = [pp.tile([128, GT * 256], bf16, tag=f"dd4{b}", name=f"dd4{b}")
               for b in range(2)]
        dd2 = [pp.tile([128, GT * 128], bf16, tag=f"dd2{b}", name=f"dd2{b}")
               for b in range(2)]
        dd = [pp.tile([128, GT * 64], bf16, tag=f"dd{b}", name=f"dd{b}")
              for b in range(2)]
        sq = [pp.tile([128, GT * 64], bf16, tag=f"sq{b}", name=f"sq{b}")
              for b in range(2)]

        def load(g):
            b = g % 2
            nc.sync.dma_start(st[b][:], stream_d[:, ds(g * G_COLS, G_COLS)])

        def compute(g):
            b = g % 2
            xc = st[b][:, 0:XC_COLS]
            mc = st[b][:, XC_COLS:G_COLS]
            p = prod[b][:]
            # prod[t, i, f, jj] = xc[t, (i), f, jj] * mc[t, i, (f), jj]
            out5 = AP(p.tensor, p.offset,
                      [p.ap[0], [512, GT], [128, 4], [8, 16], [1, 8]])
            in_x = AP(xc.tensor, xc.offset,
                      [xc.ap[0], [128, GT], [0, 4], [8, 16], [1, 8]])
            in_m = AP(mc.tensor, mc.offset,
                      [mc.ap[0], [32, GT], [8, 4], [0, 16], [1, 8]])
            nc.vector.tensor_tensor(out5, in_x, in_m, Op.mult)

            # dd4[(ti), f, jj4] = prod[.., 0:4] + prod[.., 4:8]
            a4 = dd4[b][:]
            pin0 = AP(p.tensor, p.offset,
                      [p.ap[0], [128, 4 * GT], [8, 16], [1, 4]])
            pin1 = AP(p.tensor, p.offset + 4,
                      [p.ap[0], [128, 4 * GT], [8, 16], [1, 4]])
            o4 = AP(a4.tensor, a4.offset,
                    [a4.ap[0], [64, 4 * GT], [4, 16], [1, 4]])
            nc.vector.tensor_tensor(o4, pin0, pin1, Op.add)

            # dd2 = dd4[.., 0:2] + dd4[.., 2:4]
            a2 = dd2[b][:]
            q0 = AP(a4.tensor, a4.offset,
                    [a4.ap[0], [64, 4 * GT], [4, 16], [1, 2]])
            q1 = AP(a4.tensor, a4.offset + 2,
                    [a4.ap[0], [64, 4 * GT], [4, 16], [1, 2]])
            o2 = AP(a2.tensor, a2.offset,
                    [a2.ap[0], [32, 4 * GT], [2, 16], [1, 2]])
            nc.vector.tensor_tensor(o2, q0, q1, Op.add)

            # dd = dd2[.., 0] + dd2[.., 1]  (DVE; keeping Pool idle avoids
            # SBUF-port contention that otherwise stalls the DVE ~20%)
            a1 = dd[b][:]
            r0 = AP(a2.tensor, a2.offset,
                    [a2.ap[0], [32, 4 * GT], [2, 16]])
            r1 = AP(a2.tensor, a2.offset + 1,
                    [a2.ap[0], [32, 4 * GT], [2, 16]])
            o1 = AP(a1.tensor, a1.offset,
                    [a1.ap[0], [16, 4 * GT], [1, 16]])
            nc.vector.tensor_tensor(o1, r0, r1, Op.add)

            # acc[:, g] = sum_if dd^2   (ScalarE)
            nc.scalar.activation(sq[b][:], a1, Act.Square,
                                 accum_out=acc[:, g:g + 1])

        load(0)
        for g in range(NG):
            if g + 1 < NG:
                load(g + 1)
            compute(g)

        colsum = pp.tile([128, 1], f32, tag="colsum")
        ones = pp.tile([128, 1], f32, tag="ones")
        nc.vector.reduce_sum(out=colsum[:], in_=acc[:],
                             axis=mybir.AxisListType.X)
        nc.vector.memset(ones[:], 1.0)
        with tc.tile_pool(name="psum", bufs=1, space="PSUM") as psp:
            pt = psp.tile([1, 1], f32, tag="pt")
            nc.tensor.matmul(pt[:], lhsT=colsum[:], rhs=ones[:],
                             start=True, stop=True)
            lsb = pp.tile([1, 1], f32, tag="lsb")
            # *2: each undirected pair contributes both directed edges equally
            nc.vector.tensor_scalar(lsb[:], pt[:], 2.0, None, Op.mult)
            nc.sync.dma_start(loss_d[:], lsb[:])

    nc.compile()
    return nc


_CACHED = {}


def _get_program():
    if "nc" not in _CACHED:
        _inject_axon_hooks()
        _CACHED["nc"] = _build_program()
    return _CACHED["nc"]


def _prep_core_inputs(x_bf, mapsA_bf, mapsBn_bf, dst, src, core):
    """Build the per-core bf16 stream (indexing / layout only).

    x_bf:     [N, D, F] bf16 node features
    mapsA_bf: [H, D, D] bf16 = maps[H:] (the A = F_{v->u} map of pair e)
    mapsBn_bf:[H, D, D] bf16 = -maps[:H] (negated B, sign bit flipped)
    """
    import ml_dtypes
    BF = ml_dtypes.bfloat16

    e0 = core * EPC
    e1 = e0 + EPC

    di = np.zeros(EPC_PAD, np.int64)
    di[:EPC] = dst[e0:e1]
    si = np.zeros(EPC_PAD, np.int64)
    si[:EPC] = src[e0:e1]

    # xc[e, f, jj]: jj<4 -> x[dst][jj, f], jj>=4 -> x[src][jj-4, f]
    xc = np.concatenate([x_bf[di], x_bf[si]], axis=1)   # [P, 8(jj), 16(f)]
    xc = np.ascontiguousarray(xc.transpose(0, 2, 1))    # [P, f, jj]
    xc[EPC:] = 0
    xc = xc.reshape(NG, GT * 128, DF * 2)

    # mc[e, i, jj] = [A[i, :] | -B[i, :]]
    mc = np.zeros((EPC_PAD, D, 2 * D), BF)
    mc[:EPC, :, :D] = mapsA_bf[e0:e1]
    mc[:EPC, :, D:] = mapsBn_bf[e0:e1]
    mc = mc.reshape(NG, GT * 128, 4 * 2 * D)

    # per group: [xc tiles | mc tiles], tile-major [128, cols] per tile
    stream = np.empty((128, NG, G_COLS), BF)
    stream[:, :, :XC_COLS] = (
        xc.reshape(NG, GT, 128, 128).transpose(2, 0, 1, 3).reshape(128, NG, -1))
    stream[:, :, XC_COLS:] = (
        mc.reshape(NG, GT, 128, 32).transpose(2, 0, 1, 3).reshape(128, NG, -1))
    return {"stream": np.ascontiguousarray(stream.reshape(128, NG * G_COLS))}


def _prep_all_in_maps(x, restriction_maps, edge_index):
    import ml_dtypes
    BF = ml_dtypes.bfloat16

    x_bf = np.ascontiguousarray(x.reshape(N, D, F)).astype(BF)
    maps = np.asarray(restriction_maps)
    mapsA_bf = maps[H:].astype(BF)
    mapsBn_bf = (-maps[:H]).astype(BF)
    src = np.asarray(edge_index[0], np.int64)
    dst = np.asarray(edge_index[1], np.int64)
    return [_prep_core_inputs(x_bf, mapsA_bf, mapsBn_bf, dst, src, c)
            for c in range(NCORES)]


def _symmetric_structure(rev_idx):
    r = np.asarray(rev_idx)
    if r.shape != (E,):
        return False
    h = np.arange(H, dtype=r.dtype)
    return bool(np.array_equal(r[:H], h + H) and np.array_equal(r[H:], h))


def _fallback_numpy(x, restriction_maps, edge_index, rev_idx):
    x = np.asarray(x, np.float32)
    maps = np.asarray(restriction_maps, np.float32)
    ei = np.asarray(edge_index)
    rv = np.asarray(rev_idx)
    total = np.float64(0.0)
    chunk = 131072
    ne = ei.shape[1]
    for s in range(0, ne, chunk):
        e = min(s + chunk, ne)
        src = ei[0, s:e]
        tgt = ei[1, s:e]
        fvu = maps[rv[s:e]]
        fuv = maps[s:e]
        t1 = np.einsum("eij,ejf->eif", fvu, x[tgt])
        t2 = np.einsum("eij,ejf->eif", fuv, x[src])
        d = t1 - t2
        total += np.sum((d * d).astype(np.float64))
    return np.float32(total)


def kernel(x, restriction_maps, edge_index, rev_idx):
    x = np.asarray(x)
    restriction_maps = np.asarray(restriction_maps)
    edge_index = np.asarray(edge_index)
    rev_idx = np.asarray(rev_idx)

    if (x.shape != (N, D, F) or restriction_maps.shape != (E, D, D)
            or edge_index.shape != (2, E) or not _symmetric_structure(rev_idx)):
        return _fallback_numpy(x, restriction_maps, edge_index, rev_idx)

    from concourse.bass_utils import run_bass_kernel_spmd

    nc = _get_program()
    in_maps = _prep_all_in_maps(x, restriction_maps, edge_index)
    res = run_bass_kernel_spmd(nc, in_maps, core_ids=list(range(NCORES)))
    total = np.float32(0.0)
    for c in range(NCORES):
        total += res.results[c]["loss"][0, 0]
    return np.float32(total)


# revision 12
# speedup vs baseline: 1.9792x; 1.8360x over previous
"""Trainium2 Bass kernel for sheaf Dirichlet energy (ConsistencyBasedLaplacianBuilder).

loss = sum_e || maps[rev(e)] @ x[tgt(e)] - maps[e] @ x[src(e)] ||_F^2

Strategy (edge parallelism across 8 NeuronCores):
  The reference edge set is symmetric: edge e < H (=E/2) has its reverse at
  e + H, so the loss equals 2 * sum_{e<H} ||maps[e+H] x[dst] - maps[e] x[src]||^2.
  Each core takes a contiguous slice of the H half-edges. The host lays the
  per-edge operands out as one sequential bf16 stream (pure indexing /
  layout: per edge the two 4x4 maps -- with the second negated via sign
  flip -- and the two gathered 4x16 x rows); every float multiply/add that
  produces the loss runs on device.

  Engine assignment (per group of 2048 edges):
    partition p = (lane l, jj): l = edge slot mod 16, jj = 0..7 the
    concatenated [A | -B] contraction index.
    DVE    prod[(l,jj), i, f, e16] = mc[(l,jj), i, e16] * xc[(l,jj), f, e16]
           (single tensor_tensor, all APs step-1 innermost -> 2x bf16 mode)
    PE     dd[(l,q), c] = sum_jj prod[(l,jj), slice q]  -- 8 accumulating
           selector matmuls turn the jj-sum into a partition-axis reduction
           and spread the result over all 128 PSUM partitions
    ScalarE  acc[:, g] = sum dd^2  (Square activation + accumulator, PSUM in)
  The Pool engine stays idle on purpose: its TIE SBUF traffic stalls the
  DVE 2x pipeline. f32 accumulation in PSUM and in the Square accumulator.
  Per-core partial sums are added on the host.
"""

import sys
import types

import numpy as np

sys.path.insert(0, "/opt/trn_rl_repo")

N = 50000
D = 4
F = 16
DF = D * F            # 64 floats per node row
E = 1600000
H = E // 2            # 800000 undirected pairs
NCORES = 8
EPC = H // NCORES     # 100000 half-edges per core

GE = 2048             # edges per group: 16 lanes x 128 slots
NG = 50               # groups per core
EPC_PAD = GE * NG     # 102400 >= EPC
E16 = GE // 16        # 128 edge slots per lane
XC_COLS = F * E16     # 2048 bf16 cols per group: xc [f, e16]
MC_COLS = D * E16     # 512 bf16 cols per group: mc [i, e16]
G_COLS = XC_COLS + MC_COLS
PFD = D * F * E16     # 8192: prod cols per group [i, f, e16]
NSL = 8               # matmul col slices (partition-spread factor)
SLC = PFD // NSL      # 1024 cols per slice


def _inject_axon_hooks():
    """The container's antenv lacks axon_hooks; provide it so NTFF tracing
    (used by test.py, harmless otherwise) can register."""
    if "antenv.axon_hooks" in sys.modules:
        return
    mod = types.ModuleType("antenv.axon_hooks")
    mod._hook = None

    def set_axon_ntff_profile_hook(h):
        mod._hook = h

    def get_axon_ntff_profile_hook():
        return mod._hook

    mod.set_axon_ntff_profile_hook = set_axon_ntff_profile_hook
    mod.get_axon_ntff_profile_hook = get_axon_ntff_profile_hook
    sys.modules["antenv.axon_hooks"] = mod


def _build_program(ncores=NCORES):
    import concourse.bacc as bacc
    import concourse.bass as bass
    import concourse.tile as tile
    from concourse import mybir

    AP = bass.AP
    f32 = mybir.dt.float32
    bf16 = mybir.dt.bfloat16
    Op = mybir.AluOpType
    Act = mybir.ActivationFunctionType
    ds = bass.ds

    nc = bacc.Bacc("TRN2", target_bir_lowering=False, debug=False,
                   num_devices=ncores)

    stream_d = nc.dram_tensor("stream", [128, NG * G_COLS], bf16,
                              kind="ExternalInput")
    sel_d = nc.dram_tensor("sel", [128, NSL * 128], bf16,
                           kind="ExternalInput")
    loss_d = nc.dram_tensor("loss", [1, 1], f32, kind="ExternalOutput")

    with tile.TileContext(nc) as tc, \
         tc.tile_pool(name="persist", bufs=1) as pp, \
         tc.tile_pool(name="psum", bufs=1, space="PSUM") as psp:

        acc = pp.tile([128, 2 * NG], f32, tag="acc")
        sel_sb = pp.tile([128, NSL * 128], bf16, tag="sel_sb")
        nc.sync.dma_start(sel_sb[:], sel_d[:])

        st = [pp.tile([128, G_COLS], bf16, tag=f"st{b}", name=f"st{b}")
              for b in range(2)]
        prod = [pp.tile([128, PFD], bf16, tag=f"prod{b}", name=f"prod{b}")
                for b in range(2)]
        # one PSUM bank (512 f32) per matmul output: two halves per group
        ps = [[psp.tile([128, SLC // 2], f32, tag=f"ps{b}{h}",
                        name=f"ps{b}{h}") for h in range(2)]
              for b in range(2)]
        sq = [pp.tile([128, SLC], bf16, tag=f"sq{b}", name=f"sq{b}")
              for b in range(2)]

        def load(g):
            b = g % 2
            nc.sync.dma_start(st[b][:], stream_d[:, ds(g * G_COLS, G_COLS)])

        def compute(g):
            b = g % 2
            xc = st[b][:, 0:XC_COLS]
            mc = st[b][:, XC_COLS:G_COLS]
            p = prod[b][:]
            # prod[(l,jj), i, f, e16] = xc[(l,jj), (i), f, e16]
            #                         * mc[(l,jj), i, (f), e16]
            out3 = AP(p.tensor, p.offset,
                      [p.ap[0], [F * E16, D], [E16, F], [1, E16]])
            in_x = AP(xc.tensor, xc.offset,
                      [xc.ap[0], [0, D], [E16, F], [1, E16]])
            in_m = AP(mc.tensor, mc.offset,
                      [mc.ap[0], [E16, D], [0, F], [1, E16]])
            nc.vector.tensor_tensor(out3, in_x, in_m, Op.mult)

            # dd[(l,q), c] = sum_jj prod[(l,jj), q*SLC + c]: accumulating
            # selector matmuls (partition-axis jj reduction on the PE),
            # split into 512-col halves to fit one PSUM bank each
            half = SLC // 2
            for m in range(NSL):
                for h in range(2):
                    nc.tensor.matmul(
                        ps[b][h][:],
                        lhsT=sel_sb[:, ds(m * 128, 128)],
                        rhs=prod[b][:, ds(m * SLC + h * half, half)],
                        start=(m == 0), stop=(m == NSL - 1))

            # acc[:, 2g+h] = sum dd^2   (ScalarE, PSUM source)
            for h in range(2):
                nc.scalar.activation(sq[b][:, h * half:(h + 1) * half],
                                     ps[b][h][:], Act.Square,
                                     accum_out=acc[:, 2 * g + h:2 * g + h + 1])

        load(0)
        for g in range(NG):
            if g + 1 < NG:
                load(g + 1)
            compute(g)

        colsum = pp.tile([128, 1], f32, tag="colsum")
        ones = pp.tile([128, 1], f32, tag="ones")
        nc.vector.reduce_sum(out=colsum[:], in_=acc[:],
                             axis=mybir.AxisListType.X)
        nc.vector.memset(ones[:], 1.0)
        pt = psp.tile([1, 1], f32, tag="pt")
        nc.tensor.matmul(pt[:], lhsT=colsum[:], rhs=ones[:],
                         start=True, stop=True)
        lsb = pp.tile([1, 1], f32, tag="lsb")
        # *2: each undirected pair contributes both directed edges equally
        nc.vector.tensor_scalar(lsb[:], pt[:], 2.0, None, Op.mult)
        nc.sync.dma_start(loss_d[:], lsb[:])

    nc.compile()
    return nc


_CACHED = {}


def _get_program():
    if "nc" not in _CACHED:
        _inject_axon_hooks()
        _CACHED["nc"] = _build_program()
    return _CACHED["nc"]


def _sel_matrix():
    """sel[k=(l,jj), m*128 + p'=(l',q)] = 1 iff l==l' and q==m."""
    import ml_dtypes
    sel = np.zeros((128, NSL, 128), np.float32)
    k = np.arange(128)
    lane_k = k // 8
    for m in range(NSL):
        for l in range(16):
            sel[l * 8:(l + 1) * 8, m, l * 8 + m] = 1.0
    assert sel.sum() == 128 * NSL
    return np.ascontiguousarray(sel.reshape(128, NSL * 128)
                                .astype(ml_dtypes.bfloat16))


def _prep_core_inputs(x_bf, mapsA_bf, mapsBn_bf, dst, src, core):
    """Build the per-core bf16 stream (indexing / layout only).

    Slot s of this core = half-edge e0+s; within group g, slot
    s_local = s - g*GE has lane l = s_local % 16 and column e16
    = s_local // 16; SBUF partition = l*8 + jj.
    """
    import ml_dtypes
    BF = ml_dtypes.bfloat16

    e0 = core * EPC
    e1 = e0 + EPC

    di = np.zeros(EPC_PAD, np.int64)
    di[:EPC] = dst[e0:e1]
    si = np.zeros(EPC_PAD, np.int64)
    si[:EPC] = src[e0:e1]

    # xcval[s, jj, f]: jj<4 -> x[dst][jj, f], jj>=4 -> x[src][jj-4, f]
    xcval = np.concatenate([x_bf[di], x_bf[si]], axis=1)  # [S, 8, 16]
    xcval[EPC:] = 0

    # mcval[s, i, jj] = [A[i, :] | -B[i, :]]
    mcval = np.zeros((EPC_PAD, D, 2 * D), BF)
    mcval[:EPC, :, :D] = mapsA_bf[e0:e1]
    mcval[:EPC, :, D:] = mapsBn_bf[e0:e1]

    # -> [p=(l,jj), g, (f, e16)] and [p, g, (i, e16)]
    xr = xcval.reshape(NG, E16, 16, 8, F)        # (g, e16, l, jj, f)
    xc_t = xr.transpose(2, 3, 0, 4, 1).reshape(128, NG, XC_COLS)
    mr = mcval.reshape(NG, E16, 16, D, 8)        # (g, e16, l, i, jj)
    mc_t = mr.transpose(2, 4, 0, 3, 1).reshape(128, NG, MC_COLS)

    stream = np.empty((128, NG, G_COLS), BF)
    stream[:, :, :XC_COLS] = xc_t
    stream[:, :, XC_COLS:] = mc_t
    return {"stream": np.ascontiguousarray(stream.reshape(128, NG * G_COLS)),
            "sel": _SEL}


_SEL = None


def _prep_all_in_maps(x, restriction_maps, edge_index):
    import ml_dtypes
    BF = ml_dtypes.bfloat16
    global _SEL
    if _SEL is None:
        _SEL = _sel_matrix()

    x_bf = np.ascontiguousarray(x.reshape(N, D, F)).astype(BF)
    maps = np.asarray(restriction_maps)
    mapsA_bf = maps[H:].astype(BF)
    mapsBn_bf = (-maps[:H]).astype(BF)
    src = np.asarray(edge_index[0], np.int64)
    dst = np.asarray(edge_index[1], np.int64)
    return [_prep_core_inputs(x_bf, mapsA_bf, mapsBn_bf, dst, src, c)
            for c in range(NCORES)]


def _symmetric_structure(rev_idx):
    r = np.asarray(rev_idx)
    if r.shape != (E,):
        return False
    h = np.arange(H, dtype=r.dtype)
    return bool(np.array_equal(r[:H], h + H) and np.array_equal(r[H:], h))


def _fallback_numpy(x, restriction_maps, edge_index, rev_idx):
    x = np.asarray(x, np.float32)
    maps = np.asarray(restriction_maps, np.float32)
    ei = np.asarray(edge_index)
    rv = np.asarray(rev_idx)
    total = np.float64(0.0)
    chunk = 131072
    ne = ei.shape[1]
    for s in range(0, ne, chunk):
        e = min(s + chunk, ne)
        src = ei[0, s:e]
        tgt = ei[1, s:e]
        fvu = maps[rv[s:e]]
        fuv = maps[s:e]
        t1 = np.einsum("eij,ejf->eif", fvu, x[tgt])
        t2 = np.einsum("eij,ejf->eif", fuv, x[src])
        d = t1 - t2
        total += np.sum((d * d).astype(np.float64))
    return np.float32(total)


def kernel(x, restriction_maps, edge_index, rev_idx):
    x = np.asarray(x)
    restriction_maps = np.asarray(restriction_maps)
    edge_index = np.asarray(edge_index)
    rev_idx = np.asarray(rev_idx)

    if (x.shape != (N, D, F) or restriction_maps.shape != (E, D, D)
            or edge_index.shape != (2, E) or not _symmetric_structure(rev_idx)):
        return _fallback_numpy(x, restriction_maps, edge_index, rev_idx)

    from concourse.bass_utils import run_bass_kernel_spmd

    nc = _get_program()
    in_maps = _prep_all_in_maps(x, restriction_maps, edge_index)
    res = run_bass_kernel_spmd(nc, in_maps, core_ids=list(range(NCORES)))
    total = np.float32(0.0)
    for c in range(NCORES):
        total += res.results[c]["loss"][0, 0]
    return np.float32(total)


# revision 13
# speedup vs baseline: 2.0310x; 1.0262x over previous
"""Trainium2 Bass kernel for sheaf Dirichlet energy (ConsistencyBasedLaplacianBuilder).

loss = sum_e || maps[rev(e)] @ x[tgt(e)] - maps[e] @ x[src(e)] ||_F^2

Strategy (edge parallelism across 8 NeuronCores):
  The reference edge set is symmetric: edge e < H (=E/2) has its reverse at
  e + H, so the loss equals 2 * sum_{e<H} ||maps[e+H] x[dst] - maps[e] x[src]||^2.
  Each core takes a contiguous slice of the H half-edges. The host lays the
  per-edge operands out as one sequential bf16 stream (pure indexing /
  layout: per edge the two 4x4 maps -- with the second negated via sign
  flip -- and the two gathered 4x16 x rows); every float multiply/add that
  produces the loss runs on device.

  Engine assignment (per group of 2048 edges):
    partition p = (lane l, jj): l = edge slot mod 16, jj = 0..7 the
    concatenated [A | -B] contraction index.
    DVE    prod[(l,jj), i, f, e16] = mc[(l,jj), i, e16] * xc[(l,jj), f, e16]
           (single tensor_tensor, all APs step-1 innermost -> 2x bf16 mode)
    PE     dd[(l,q), c] = sum_jj prod[(l,jj), slice q]  -- 8 accumulating
           selector matmuls turn the jj-sum into a partition-axis reduction
           and spread the result over all 128 PSUM partitions
    ScalarE  acc[:, g] = sum dd^2  (Square activation + accumulator, PSUM in)
  The Pool engine stays idle on purpose: its TIE SBUF traffic stalls the
  DVE 2x pipeline. f32 accumulation in PSUM and in the Square accumulator.
  Per-core partial sums are added on the host.
"""

import sys
import types

import numpy as np

sys.path.insert(0, "/opt/trn_rl_repo")

N = 50000
D = 4
F = 16
DF = D * F            # 64 floats per node row
E = 1600000
H = E // 2            # 800000 undirected pairs
NCORES = 8
EPC = H // NCORES     # 100000 half-edges per core

GE = 2048             # edges per group: 16 lanes x 128 slots
NG = 50               # groups per core
EPC_PAD = GE * NG     # 102400 >= EPC
E16 = GE // 16        # 128 edge slots per lane
XC_COLS = F * E16     # 2048 bf16 cols per group: xc [f, e16]
MC_COLS = D * E16     # 512 bf16 cols per group: mc [i, e16]
G_COLS = XC_COLS + MC_COLS
PFD = D * F * E16     # 8192: prod cols per group [i, f, e16]
NSL = 8               # matmul col slices (partition-spread factor)
SLC = PFD // NSL      # 1024 cols per slice


def _inject_axon_hooks():
    """The container's antenv lacks axon_hooks; provide it so NTFF tracing
    (used by test.py, harmless otherwise) can register."""
    if "antenv.axon_hooks" in sys.modules:
        return
    mod = types.ModuleType("antenv.axon_hooks")
    mod._hook = None

    def set_axon_ntff_profile_hook(h):
        mod._hook = h

    def get_axon_ntff_profile_hook():
        return mod._hook

    mod.set_axon_ntff_profile_hook = set_axon_ntff_profile_hook
    mod.get_axon_ntff_profile_hook = get_axon_ntff_profile_hook
    sys.modules["antenv.axon_hooks"] = mod


def _build_program(ncores=NCORES):
    import concourse.bacc as bacc
    import concourse.bass as bass
    import concourse.tile as tile
    from concourse import mybir

    AP = bass.AP
    f32 = mybir.dt.float32
    bf16 = mybir.dt.bfloat16
    Op = mybir.AluOpType
    Act = mybir.ActivationFunctionType
    ds = bass.ds

    nc = bacc.Bacc("TRN2", target_bir_lowering=False, debug=False,
                   num_devices=ncores)

    stream_d = nc.dram_tensor("stream", [128, NG * G_COLS], bf16,
                              kind="ExternalInput")
    sel_d = nc.dram_tensor("sel", [128, NSL * 128], bf16,
                           kind="ExternalInput")
    loss_d = nc.dram_tensor("loss", [1, 1], f32, kind="ExternalOutput")

    with tile.TileContext(nc) as tc, \
         tc.tile_pool(name="persist", bufs=1) as pp, \
         tc.tile_pool(name="psum", bufs=1, space="PSUM") as psp:

        acc = pp.tile([128, 2 * NG], f32, tag="acc")
        sel_sb = pp.tile([128, NSL * 128], bf16, tag="sel_sb")
        nc.sync.dma_start(sel_sb[:], sel_d[:])

        NB = 3            # buffer depth: DVE never waits on PE/DMA lag
        st = [pp.tile([128, G_COLS], bf16, tag=f"st{b}", name=f"st{b}")
              for b in range(NB)]
        prod = [pp.tile([128, PFD], bf16, tag=f"prod{b}", name=f"prod{b}")
                for b in range(NB)]
        # one PSUM bank (512 f32) per matmul output: two halves per group
        ps = [[psp.tile([128, SLC // 2], f32, tag=f"ps{b}{h}",
                        name=f"ps{b}{h}") for h in range(2)]
              for b in range(NB)]
        sq = [pp.tile([128, SLC], bf16, tag=f"sq{b}", name=f"sq{b}")
              for b in range(NB)]

        def load(g):
            b = g % NB
            nc.sync.dma_start(st[b][:], stream_d[:, ds(g * G_COLS, G_COLS)])

        def compute(g):
            b = g % NB
            xc = st[b][:, 0:XC_COLS]
            mc = st[b][:, XC_COLS:G_COLS]
            p = prod[b][:]
            # prod[(l,jj), i, f, e16] = xc[(l,jj), (i), f, e16]
            #                         * mc[(l,jj), i, (f), e16]
            out3 = AP(p.tensor, p.offset,
                      [p.ap[0], [F * E16, D], [E16, F], [1, E16]])
            in_x = AP(xc.tensor, xc.offset,
                      [xc.ap[0], [0, D], [E16, F], [1, E16]])
            in_m = AP(mc.tensor, mc.offset,
                      [mc.ap[0], [E16, D], [0, F], [1, E16]])
            nc.vector.tensor_tensor(out3, in_x, in_m, Op.mult)

            # dd[(l,q), c] = sum_jj prod[(l,jj), q*SLC + c]: accumulating
            # selector matmuls (partition-axis jj reduction on the PE),
            # split into 512-col halves to fit one PSUM bank each
            half = SLC // 2
            for m in range(NSL):
                for h in range(2):
                    nc.tensor.matmul(
                        ps[b][h][:],
                        lhsT=sel_sb[:, ds(m * 128, 128)],
                        rhs=prod[b][:, ds(m * SLC + h * half, half)],
                        start=(m == 0), stop=(m == NSL - 1))

            # acc[:, 2g+h] = sum dd^2   (ScalarE, PSUM source)
            for h in range(2):
                nc.scalar.activation(sq[b][:, h * half:(h + 1) * half],
                                     ps[b][h][:], Act.Square,
                                     accum_out=acc[:, 2 * g + h:2 * g + h + 1])

        load(0)
        load(1)
        for g in range(NG):
            if g + 2 < NG:
                load(g + 2)
            compute(g)

        colsum = pp.tile([128, 1], f32, tag="colsum")
        ones = pp.tile([128, 1], f32, tag="ones")
        nc.vector.reduce_sum(out=colsum[:], in_=acc[:],
                             axis=mybir.AxisListType.X)
        nc.vector.memset(ones[:], 1.0)
        pt = psp.tile([1, 1], f32, tag="pt")
        nc.tensor.matmul(pt[:], lhsT=colsum[:], rhs=ones[:],
                         start=True, stop=True)
        lsb = pp.tile([1, 1], f32, tag="lsb")
        # *2: each undirected pair contributes both directed edges equally
        nc.vector.tensor_scalar(lsb[:], pt[:], 2.0, None, Op.mult)
        nc.sync.dma_start(loss_d[:], lsb[:])

    nc.compile()
    return nc


_CACHED = {}


def _get_program():
    if "nc" not in _CACHED:
        _inject_axon_hooks()
        _CACHED["nc"] = _build_program()
    return _CACHED["nc"]


def _sel_matrix():
    """sel[k=(l,jj), m*128 + p'=(l',q)] = 1 iff l==l' and q==m."""
    import ml_dtypes
    sel = np.zeros((128, NSL, 128), np.float32)
    k = np.arange(128)
    lane_k = k // 8
    for m in range(NSL):
        for l in range(16):
            sel[l * 8:(l + 1) * 8, m, l * 8 + m] = 1.0
    assert sel.sum() == 128 * NSL
    return np.ascontiguousarray(sel.reshape(128, NSL * 128)
                                .astype(ml_dtypes.bfloat16))


def _prep_core_inputs(x_bf, mapsA_bf, mapsBn_bf, dst, src, core):
    """Build the per-core bf16 stream (indexing / layout only).

    Slot s of this core = half-edge e0+s; within group g, slot
    s_local = s - g*GE has lane l = s_local % 16 and column e16
    = s_local // 16; SBUF partition = l*8 + jj.
    """
    import ml_dtypes
    BF = ml_dtypes.bfloat16

    e0 = core * EPC
    e1 = e0 + EPC

    di = np.zeros(EPC_PAD, np.int64)
    di[:EPC] = dst[e0:e1]
    si = np.zeros(EPC_PAD, np.int64)
    si[:EPC] = src[e0:e1]

    # xcval[s, jj, f]: jj<4 -> x[dst][jj, f], jj>=4 -> x[src][jj-4, f]
    xcval = np.concatenate([x_bf[di], x_bf[si]], axis=1)  # [S, 8, 16]
    xcval[EPC:] = 0

    # mcval[s, i, jj] = [A[i, :] | -B[i, :]]
    mcval = np.zeros((EPC_PAD, D, 2 * D), BF)
    mcval[:EPC, :, :D] = mapsA_bf[e0:e1]
    mcval[:EPC, :, D:] = mapsBn_bf[e0:e1]

    # -> [p=(l,jj), g, (f, e16)] and [p, g, (i, e16)]
    xr = xcval.reshape(NG, E16, 16, 8, F)        # (g, e16, l, jj, f)
    xc_t = xr.transpose(2, 3, 0, 4, 1).reshape(128, NG, XC_COLS)
    mr = mcval.reshape(NG, E16, 16, D, 8)        # (g, e16, l, i, jj)
    mc_t = mr.transpose(2, 4, 0, 3, 1).reshape(128, NG, MC_COLS)

    stream = np.empty((128, NG, G_COLS), BF)
    stream[:, :, :XC_COLS] = xc_t
    stream[:, :, XC_COLS:] = mc_t
    return {"stream": np.ascontiguousarray(stream.reshape(128, NG * G_COLS)),
            "sel": _SEL}


_SEL = None


def _prep_all_in_maps(x, restriction_maps, edge_index):
    import ml_dtypes
    BF = ml_dtypes.bfloat16
    global _SEL
    if _SEL is None:
        _SEL = _sel_matrix()

    x_bf = np.ascontiguousarray(x.reshape(N, D, F)).astype(BF)
    maps = np.asarray(restriction_maps)
    mapsA_bf = maps[H:].astype(BF)
    mapsBn_bf = (-maps[:H]).astype(BF)
    src = np.asarray(edge_index[0], np.int64)
    dst = np.asarray(edge_index[1], np.int64)
    return [_prep_core_inputs(x_bf, mapsA_bf, mapsBn_bf, dst, src, c)
            for c in range(NCORES)]


def _symmetric_structure(rev_idx):
    r = np.asarray(rev_idx)
    if r.shape != (E,):
        return False
    h = np.arange(H, dtype=r.dtype)
    return bool(np.array_equal(r[:H], h + H) and np.array_equal(r[H:], h))


def _fallback_numpy(x, restriction_maps, edge_index, rev_idx):
    x = np.asarray(x, np.float32)
    maps = np.asarray(restriction_maps, np.float32)
    ei = np.asarray(edge_index)
    rv = np.asarray(rev_idx)
    total = np.float64(0.0)
    chunk = 131072
    ne = ei.shape[1]
    for s in range(0, ne, chunk):
        e = min(s + chunk, ne)
        src = ei[0, s:e]
        tgt = ei[1, s:e]
        fvu = maps[rv[s:e]]
        fuv = maps[s:e]
        t1 = np.einsum("eij,ejf->eif", fvu, x[tgt])
        t2 = np.einsum("eij,ejf->eif", fuv, x[src])
        d = t1 - t2
        total += np.sum((d * d).astype(np.float64))
    return np.float32(total)


def kernel(x, restriction_maps, edge_index, rev_idx):
    x = np.asarray(x)
    restriction_maps = np.asarray(restriction_maps)
    edge_index = np.asarray(edge_index)
    rev_idx = np.asarray(rev_idx)

    if (x.shape != (N, D, F) or restriction_maps.shape != (E, D, D)
            or edge_index.shape != (2, E) or not _symmetric_structure(rev_idx)):
        return _fallback_numpy(x, restriction_maps, edge_index, rev_idx)

    from concourse.bass_utils import run_bass_kernel_spmd

    nc = _get_program()
    in_maps = _prep_all_in_maps(x, restriction_maps, edge_index)
    res = run_bass_kernel_spmd(nc, in_maps, core_ids=list(range(NCORES)))
    total = np.float32(0.0)
    for c in range(NCORES):
        total += res.results[c]["loss"][0, 0]
    return np.float32(total)


# revision 14
# speedup vs baseline: 2.0371x; 1.0030x over previous
"""Trainium2 Bass kernel for sheaf Dirichlet energy (ConsistencyBasedLaplacianBuilder).

loss = sum_e || maps[rev(e)] @ x[tgt(e)] - maps[e] @ x[src(e)] ||_F^2

Strategy (edge parallelism across 8 NeuronCores):
  The reference edge set is symmetric: edge e < H (=E/2) has its reverse at
  e + H, so the loss equals 2 * sum_{e<H} ||maps[e+H] x[dst] - maps[e] x[src]||^2.
  Each core takes a contiguous slice of the H half-edges. The host lays the
  per-edge operands out as one sequential bf16 stream (pure indexing /
  layout: per edge the two 4x4 maps -- with the second negated via sign
  flip -- and the two gathered 4x16 x rows); every float multiply/add that
  produces the loss runs on device.

  Engine assignment (per group of 2048 edges):
    partition p = (lane l, jj): l = edge slot mod 16, jj = 0..7 the
    concatenated [A | -B] contraction index.
    DVE    prod[(l,jj), i, f, e16] = mc[(l,jj), i, e16] * xc[(l,jj), f, e16]
           (single tensor_tensor, all APs step-1 innermost -> 2x bf16 mode)
    PE     dd[(l,q), c] = sum_jj prod[(l,jj), slice q]  -- 8 accumulating
           selector matmuls turn the jj-sum into a partition-axis reduction
           and spread the result over all 128 PSUM partitions
    ScalarE  acc[:, g] = sum dd^2  (Square activation + accumulator, PSUM in)
  The Pool engine stays idle on purpose: its TIE SBUF traffic stalls the
  DVE 2x pipeline. f32 accumulation in PSUM and in the Square accumulator.
  Per-core partial sums are added on the host.
"""

import sys
import types

import numpy as np

sys.path.insert(0, "/opt/trn_rl_repo")

N = 50000
D = 4
F = 16
DF = D * F            # 64 floats per node row
E = 1600000
H = E // 2            # 800000 undirected pairs
NCORES = 8
EPC = H // NCORES     # 100000 half-edges per core

# Group plan: 512-edge mini groups at both ends shorten the first-DMA
# ramp and the trailing PE/Act drain; 2048-edge groups in the middle.
GROUPS = [512] * 4 + [2048] * 48 + [512] * 4
EPC_PAD = sum(GROUPS)  # 102400 >= EPC
NGT = len(GROUPS)
GE_MAX = max(GROUPS)
E16_MAX = GE_MAX // 16
NSL = 8               # matmul col slices (partition-spread factor)


def _gplan():
    """(ge, slot0, col0) per group; cols per group = ge//16*(F+D)."""
    out = []
    s0 = c0 = 0
    for ge in GROUPS:
        out.append((ge, s0, c0))
        s0 += ge
        c0 += (ge // 16) * (F + D)
    return out


GPLAN = _gplan()
TOT_COLS = GPLAN[-1][2] + (GROUPS[-1] // 16) * (F + D)


def _inject_axon_hooks():
    """The container's antenv lacks axon_hooks; provide it so NTFF tracing
    (used by test.py, harmless otherwise) can register."""
    if "antenv.axon_hooks" in sys.modules:
        return
    mod = types.ModuleType("antenv.axon_hooks")
    mod._hook = None

    def set_axon_ntff_profile_hook(h):
        mod._hook = h

    def get_axon_ntff_profile_hook():
        return mod._hook

    mod.set_axon_ntff_profile_hook = set_axon_ntff_profile_hook
    mod.get_axon_ntff_profile_hook = get_axon_ntff_profile_hook
    sys.modules["antenv.axon_hooks"] = mod


def _build_program(ncores=NCORES):
    import concourse.bacc as bacc
    import concourse.bass as bass
    import concourse.tile as tile
    from concourse import mybir

    AP = bass.AP
    f32 = mybir.dt.float32
    bf16 = mybir.dt.bfloat16
    Op = mybir.AluOpType
    Act = mybir.ActivationFunctionType
    ds = bass.ds

    nc = bacc.Bacc("TRN2", target_bir_lowering=False, debug=False,
                   num_devices=ncores)

    stream_d = nc.dram_tensor("stream", [128, TOT_COLS], bf16,
                              kind="ExternalInput")
    sel_d = nc.dram_tensor("sel", [128, NSL * 128], bf16,
                           kind="ExternalInput")
    loss_d = nc.dram_tensor("loss", [1, 1], f32, kind="ExternalOutput")

    with tile.TileContext(nc) as tc, \
         tc.tile_pool(name="persist", bufs=1) as pp, \
         tc.tile_pool(name="psum", bufs=1, space="PSUM") as psp:

        acc = pp.tile([128, 2 * NGT], f32, tag="acc")
        sel_sb = pp.tile([128, NSL * 128], bf16, tag="sel_sb")
        nc.sync.dma_start(sel_sb[:], sel_d[:])

        NB = 3            # buffer depth: DVE never waits on PE/DMA lag
        st = [pp.tile([128, E16_MAX * (F + D)], bf16, tag=f"st{b}", name=f"st{b}")
              for b in range(NB)]
        prod = [pp.tile([128, D * F * E16_MAX], bf16, tag=f"prod{b}", name=f"prod{b}")
                for b in range(NB)]
        # one PSUM bank (512 f32) per matmul output: two halves per group
        ps = [[psp.tile([128, D * F * E16_MAX // NSL // 2], f32,
                        tag=f"ps{b}{h}", name=f"ps{b}{h}") for h in range(2)]
              for b in range(NB)]
        sq = [pp.tile([128, D * F * E16_MAX // NSL], bf16, tag=f"sq{b}",
              name=f"sq{b}") for b in range(NB)]

        def load(g):
            b = g % NB
            ge, _, col0 = GPLAN[g]
            cols = (ge // 16) * (F + D)
            nc.sync.dma_start(st[b][:, 0:cols], stream_d[:, ds(col0, cols)])

        def compute(g):
            b = g % NB
            ge, _, _ = GPLAN[g]
            e16 = ge // 16
            xc_cols = F * e16
            pfd = D * F * e16
            slc = pfd // NSL
            half = slc // 2
            xc = st[b][:, 0:xc_cols]
            mc = st[b][:, xc_cols:xc_cols + D * e16]
            p = prod[b][:]
            # prod[(l,jj), i, f, e16] = xc[(l,jj), (i), f, e16]
            #                         * mc[(l,jj), i, (f), e16]
            out3 = AP(p.tensor, p.offset,
                      [p.ap[0], [F * e16, D], [e16, F], [1, e16]])
            in_x = AP(xc.tensor, xc.offset,
                      [xc.ap[0], [0, D], [e16, F], [1, e16]])
            in_m = AP(mc.tensor, mc.offset,
                      [mc.ap[0], [e16, D], [0, F], [1, e16]])
            nc.vector.tensor_tensor(out3, in_x, in_m, Op.mult)

            # dd[(l,q), c] = sum_jj prod[(l,jj), q*slc + c]: accumulating
            # selector matmuls (partition-axis jj reduction on the PE),
            # split into halves to fit one PSUM bank each
            for m in range(NSL):
                for h in range(2):
                    nc.tensor.matmul(
                        ps[b][h][:, 0:half],
                        lhsT=sel_sb[:, ds(m * 128, 128)],
                        rhs=prod[b][:, ds(m * slc + h * half, half)],
                        start=(m == 0), stop=(m == NSL - 1))

            # acc[:, 2g+h] = sum dd^2   (ScalarE, PSUM source)
            for h in range(2):
                nc.scalar.activation(sq[b][:, h * half:(h + 1) * half],
                                     ps[b][h][:, 0:half], Act.Square,
                                     accum_out=acc[:, 2 * g + h:2 * g + h + 1])

        load(0)
        load(1)
        for g in range(NGT):
            if g + 2 < NGT:
                load(g + 2)
            compute(g)

        colsum = pp.tile([128, 1], f32, tag="colsum")
        ones = pp.tile([128, 1], f32, tag="ones")
        nc.vector.reduce_sum(out=colsum[:], in_=acc[:],
                             axis=mybir.AxisListType.X)
        nc.vector.memset(ones[:], 1.0)
        pt = psp.tile([1, 1], f32, tag="pt")
        nc.tensor.matmul(pt[:], lhsT=colsum[:], rhs=ones[:],
                         start=True, stop=True)
        lsb = pp.tile([1, 1], f32, tag="lsb")
        # *2: each undirected pair contributes both directed edges equally
        nc.vector.tensor_scalar(lsb[:], pt[:], 2.0, None, Op.mult)
        nc.sync.dma_start(loss_d[:], lsb[:])

    nc.compile()
    return nc


_CACHED = {}


def _get_program():
    if "nc" not in _CACHED:
        _inject_axon_hooks()
        _CACHED["nc"] = _build_program()
    return _CACHED["nc"]


def _sel_matrix():
    """sel[k=(l,jj), m*128 + p'=(l',q)] = 1 iff l==l' and q==m."""
    import ml_dtypes
    sel = np.zeros((128, NSL, 128), np.float32)
    k = np.arange(128)
    lane_k = k // 8
    for m in range(NSL):
        for l in range(16):
            sel[l * 8:(l + 1) * 8, m, l * 8 + m] = 1.0
    assert sel.sum() == 128 * NSL
    return np.ascontiguousarray(sel.reshape(128, NSL * 128)
                                .astype(ml_dtypes.bfloat16))


def _prep_core_inputs(x_bf, mapsA_bf, mapsBn_bf, dst, src, core):
    """Build the per-core bf16 stream (indexing / layout only).

    Slot s of this core = half-edge e0+s; within group g, slot
    s_local = s - g*GE has lane l = s_local % 16 and column e16
    = s_local // 16; SBUF partition = l*8 + jj.
    """
    import ml_dtypes
    BF = ml_dtypes.bfloat16

    e0 = core * EPC
    e1 = e0 + EPC

    di = np.zeros(EPC_PAD, np.int64)
    di[:EPC] = dst[e0:e1]
    si = np.zeros(EPC_PAD, np.int64)
    si[:EPC] = src[e0:e1]

    # xcval[s, jj, f]: jj<4 -> x[dst][jj, f], jj>=4 -> x[src][jj-4, f]
    xcval = np.concatenate([x_bf[di], x_bf[si]], axis=1)  # [S, 8, 16]
    xcval[EPC:] = 0

    # mcval[s, i, jj] = [A[i, :] | -B[i, :]]
    mcval = np.zeros((EPC_PAD, D, 2 * D), BF)
    mcval[:EPC, :, :D] = mapsA_bf[e0:e1]
    mcval[:EPC, :, D:] = mapsBn_bf[e0:e1]

    # -> per group: [p=(l,jj), (f, e16)] ++ [p, (i, e16)]
    stream = np.empty((128, TOT_COLS), BF)
    for ge, s0, c0 in GPLAN:
        e16 = ge // 16
        xr = xcval[s0:s0 + ge].reshape(e16, 16, 8, F)   # (e16, l, jj, f)
        xc_t = xr.transpose(1, 2, 3, 0).reshape(128, F * e16)
        mr = mcval[s0:s0 + ge].reshape(e16, 16, D, 8)   # (e16, l, i, jj)
        mc_t = mr.transpose(1, 3, 2, 0).reshape(128, D * e16)
        stream[:, c0:c0 + F * e16] = xc_t
        stream[:, c0 + F * e16:c0 + (F + D) * e16] = mc_t
    return {"stream": np.ascontiguousarray(stream), "sel": _SEL}


_SEL = None


def _prep_all_in_maps(x, restriction_maps, edge_index):
    import ml_dtypes
    BF = ml_dtypes.bfloat16
    global _SEL
    if _SEL is None:
        _SEL = _sel_matrix()

    x_bf = np.ascontiguousarray(x.reshape(N, D, F)).astype(BF)
    maps = np.asarray(restriction_maps)
    mapsA_bf = maps[H:].astype(BF)
    mapsBn_bf = (-maps[:H]).astype(BF)
    src = np.asarray(edge_index[0], np.int64)
    dst = np.asarray(edge_index[1], np.int64)
    return [_prep_core_inputs(x_bf, mapsA_bf, mapsBn_bf, dst, src, c)
            for c in range(NCORES)]


def _symmetric_structure(rev_idx):
    r = np.asarray(rev_idx)
    if r.shape != (E,):
        return False
    h = np.arange(H, dtype=r.dtype)
    return bool(np.array_equal(r[:H], h + H) and np.array_equal(r[H:], h))


def _fallback_numpy(x, restriction_maps, edge_index, rev_idx):
    x = np.asarray(x, np.float32)
    maps = np.asarray(restriction_maps, np.float32)
    ei = np.asarray(edge_index)
    rv = np.asarray(rev_idx)
    total = np.float64(0.0)
    chunk = 131072
    ne = ei.shape[1]
    for s in range(0, ne, chunk):
        e = min(s + chunk, ne)
        src = ei[0, s:e]
        tgt = ei[1, s:e]
        fvu = maps[rv[s:e]]
        fuv = maps[s:e]
        t1 = np.einsum("eij,ejf->eif", fvu, x[tgt])
        t2 = np.einsum("eij,ejf->eif", fuv, x[src])
        d = t1 - t2
        total += np.sum((d * d).astype(np.float64))
    return np.float32(total)


def kernel(x, restriction_maps, edge_index, rev_idx):
    x = np.asarray(x)
    restriction_maps = np.asarray(restriction_maps)
    edge_index = np.asarray(edge_index)
    rev_idx = np.asarray(rev_idx)

    if (x.shape != (N, D, F) or restriction_maps.shape != (E, D, D)
            or edge_index.shape != (2, E) or not _symmetric_structure(rev_idx)):
        return _fallback_numpy(x, restriction_maps, edge_index, rev_idx)

    from concourse.bass_utils import run_bass_kernel_spmd

    nc = _get_program()
    in_maps = _prep_all_in_maps(x, restriction_maps, edge_index)
    res = run_bass_kernel_spmd(nc, in_maps, core_ids=list(range(NCORES)))
    total = np.float32(0.0)
    for c in range(NCORES):
        total += res.results[c]["loss"][0, 0]
    return np.float32(total)
